# revision 9
# baseline (speedup 1.0000x reference)
"""Single-head attention (B=4, N=4096, H=768, D=64) on 8 TRN2 NeuronCores.

Sharding: core = (batch, query-half). Each core receives the full batch's
x rows (rotated so its 2048 query rows come first -- softmax over keys is
permutation invariant), computes K/V for all 4096 keys and attention for
its 2048 queries. Output [2048, 64] per core, reassembled on the host.
"""

import os
import sys

sys.path.insert(0, "/opt/trn_rl_repo")

import numpy as np

import concourse.bass as bass
import concourse.tile as tile
from concourse import bacc, mybir
from concourse.bass_utils import run_bass_kernel_spmd
from concourse.masks import make_identity

B = 4
N = 4096          # keys per batch
NQ = 2048         # queries per core
H = 768
D = 64
P = 128
HC = H // P       # 6 contraction chunks
NKB = N // P      # 32 key blocks
NQB = NQ // 512   # 4 query col-blocks
NTB = N // 512    # 8 token col-blocks for K/V projections
NCORES = 8

DT = mybir.dt.float32
FDT = mybir.dt.float32r  # fast fp32 streaming mode on the PE array

AF = mybir.ActivationFunctionType


def _mm(ap):
    """View an fp32 AP as float32r for full-rate PE streaming."""
    return ap.bitcast(FDT)


# Matmul-feeding SBUF tiles are declared float32r (walrus requires the
# producer instruction to emit f32r); transposes run in plain f32.


def _attention_head(ctx, tc, out, x, Ws, biases):
    nc = tc.nc
    Wq, Wk, Wv = Ws
    bq, bk, bv = biases

    const = ctx.enter_context(tc.tile_pool(name="const", bufs=1))
    big = ctx.enter_context(tc.tile_pool(name="big", bufs=1))
    xin = ctx.enter_context(tc.tile_pool(name="xin", bufs=3))
    psA = ctx.enter_context(tc.tile_pool(name="psA", bufs=2, space="PSUM"))
    psS = ctx.enter_context(tc.tile_pool(name="psS", bufs=2, space="PSUM"))
    psO = ctx.enter_context(tc.tile_pool(name="psO", bufs=4, space="PSUM"))
    sbE = ctx.enter_context(tc.tile_pool(name="sbE", bufs=3))
    sbo = ctx.enter_context(tc.tile_pool(name="sbo", bufs=4))

    # --- constants -----------------------------------------------------
    ident = const.tile([P, P], DT)
    make_identity(nc, ident)

    w_sb = const.tile([P, 3 * HC * D], FDT)  # [128, 1152]: q,k,v chunks
    for wi, w in enumerate((Wq, Wk, Wv)):
        for c in range(HC):
            nc.sync.dma_start(
                w_sb[:, (wi * HC + c) * D : (wi * HC + c + 1) * D],
                w[c * P : (c + 1) * P, :].bitcast(FDT),
            )
    bias_sb = const.tile([D, 3], DT)
    for bi, b in enumerate((bq, bk, bv)):
        nc.sync.dma_start(
            bias_sb[:, bi : bi + 1], b.rearrange("(p f) -> p f", f=1)
        )

    # --- phase A: xT = x.T into SBUF, [128, HC*4096] ------------------
    xT = big.tile([P, HC * N], FDT)
    for nb in range(NKB):
        xt = xin.tile([P, H], DT)
        nc.sync.dma_start(xt[:], x[nb * P : (nb + 1) * P, :])
        for c in range(HC):
            pt = psA.tile([P, P], DT, tag="mm")
            nc.tensor.transpose(pt[:], xt[:, c * P : (c + 1) * P], ident[:])
            nc.vector.tensor_copy(
                xT[:, c * N + nb * P : c * N + (nb + 1) * P], pt[:]
            )

    # --- phase B: projections -----------------------------------------
    # KT [64, 4096], QT [64, 2048], VTa [65, 4096] (row 64 = ones)
    kT = big.tile([D, N], FDT)
    qT = big.tile([D, NQ], FDT)
    vTa = big.tile([D + 1, N], DT)

    def project(dst, wi, tb, bias_col):
        ps = psA.tile([D, 512], DT, tag="mm")
        for c in range(HC):
            nc.tensor.matmul(
                ps[:],
                w_sb[:, (wi * HC + c) * D : (wi * HC + c + 1) * D],
                xT[:, c * N + tb * 512 : c * N + tb * 512 + 512],
                start=(c == 0),
                stop=(c == HC - 1),
            )
        nc.scalar.activation(
            dst[0:D, tb * 512 : (tb + 1) * 512],
            ps[:],
            AF.Identity,
            bias=bias_sb[:, bias_col : bias_col + 1],
        )

    for tb in range(NTB):
        project(kT, 1, tb, 1)
    for tb in range(NTB):
        project(vTa, 2, tb, 2)
    for tb in range(NQB):
        project(qT, 0, tb, 0)
    nc.gpsimd.memset(vTa[D : D + 1, :], 1.0)

    # --- phase D: V' blocks [128, 65] per key block -------------------
    v_sb = big.tile([P, NKB * (D + 1)], FDT)
    for kb in range(NKB):
        pt = psA.tile([P, D + 1], DT, tag="mm")
        nc.tensor.transpose(
            pt[:],
            vTa[:, kb * P : (kb + 1) * P],
            ident[: D + 1, : D + 1],
        )
        nc.vector.tensor_copy(v_sb[:, kb * (D + 1) : (kb + 1) * (D + 1)], pt[:])

    # --- phase E: attention -------------------------------------------
    att = [
        psO.tile([D + 1, 512], DT, name=f"att{qb}", tag="att")
        for qb in range(NQB)
    ]
    for kb in range(NKB):
        for qb in range(NQB):
            ps = psS.tile([P, 512], DT, tag="sc")
            nc.tensor.matmul(
                ps[:],
                kT[:, kb * P : (kb + 1) * P],
                qT[:, qb * 512 : (qb + 1) * 512],
                start=True,
                stop=True,
            )
            ex = sbE.tile([P, 512], FDT)
            nc.scalar.activation(ex[:], ps[:], AF.Exp, scale=float(D) ** -0.5)
            nc.tensor.matmul(
                att[qb][:],
                v_sb[:, kb * (D + 1) : (kb + 1) * (D + 1)],
                ex[:],
                start=(kb == 0),
                stop=(kb == NKB - 1),
            )

    # --- phase F: normalize + output ----------------------------------
    for qb in range(NQB):
        asb = sbo.tile([D + 1, 512], DT, tag="asb")
        nc.vector.tensor_copy(asb[:], att[qb][:])
        for sub in range(4):
            pt = psA.tile([P, D + 1], DT, tag="mm")
            nc.tensor.transpose(
                pt[:],
                asb[:, sub * P : (sub + 1) * P],
                ident[: D + 1, : D + 1],
            )
            rc = sbo.tile([P, 1], DT, tag="rc")
            nc.vector.reciprocal(rc[:], pt[:, D : D + 1])
            ob = sbo.tile([P, D], DT, tag="ob")
            nc.vector.tensor_scalar_mul(ob[:], pt[:, 0:D], rc[:])
            r0 = (qb * 4 + sub) * P
            nc.sync.dma_start(out[r0 : r0 + P, :], ob[:])


_NC_CACHE = None


def _build():
    global _NC_CACHE
    if _NC_CACHE is not None:
        return _NC_CACHE
    nc = bacc.Bacc(
        "TRN2",
        target_bir_lowering=False,
        debug=False,
        enable_asserts=True,
        num_devices=NCORES,
    )
    x = nc.dram_tensor("x", [N, H], DT, kind="ExternalInput").ap()
    Wq = nc.dram_tensor("Wq", [H, D], DT, kind="ExternalInput").ap()
    bq = nc.dram_tensor("bq", [D], DT, kind="ExternalInput").ap()
    Wk = nc.dram_tensor("Wk", [H, D], DT, kind="ExternalInput").ap()
    bk = nc.dram_tensor("bk", [D], DT, kind="ExternalInput").ap()
    Wv = nc.dram_tensor("Wv", [H, D], DT, kind="ExternalInput").ap()
    bv = nc.dram_tensor("bv", [D], DT, kind="ExternalInput").ap()
    out = nc.dram_tensor("out", [NQ, D], DT, kind="ExternalOutput").ap()

    from contextlib import ExitStack

    with tile.TileContext(nc) as tc:
        with ExitStack() as ctx:
            _attention_head(ctx, tc, out, x, (Wq, Wk, Wv), (bq, bk, bv))
    nc.compile()
    _NC_CACHE = nc
    return nc


def _make_in_maps(inputs):
    x = np.ascontiguousarray(np.asarray(inputs["x"], dtype=np.float32))
    small = {
        k: np.ascontiguousarray(np.asarray(inputs[k], dtype=np.float32))
        for k in ("Wq", "bq", "Wk", "bk", "Wv", "bv")
    }
    in_maps = []
    for core in range(NCORES):
        b, h = divmod(core, 2)
        xb = x[b]
        if h == 1:
            xb = np.concatenate([xb[NQ:], xb[:NQ]], axis=0)
        in_maps.append({"x": np.ascontiguousarray(xb), **small})
    return in_maps


def _run(inputs, trace=False):
    nc = _build()
    res = run_bass_kernel_spmd(
        nc, _make_in_maps(inputs), core_ids=list(range(NCORES)), trace=trace
    )
    out = np.empty((B, N, D), dtype=np.float32)
    for core in range(NCORES):
        b, h = divmod(core, 2)
        out[b, h * NQ : (h + 1) * NQ] = res.results[core]["out"]
    return out, res


def kernel(**inputs):
    out, _ = _run(inputs, trace=False)
    return out


def _install_ntff_hook():
    """Register the axon NTFF profiling hook that this image's antenv lacks."""
    import types

    try:
        import antenv.axon_hooks  # noqa: F401

        return
    except ImportError:
        pass
    import antenv
    from trn_agent_boot.trn_boot import _ntff_profile_via_ctypes

    import concourse.bass_utils as bu

    mod = types.ModuleType("antenv.axon_hooks")
    _h = [None]
    mod.set_axon_ntff_profile_hook = lambda h: _h.__setitem__(0, h)
    mod.get_axon_ntff_profile_hook = lambda: _h[0]
    sys.modules["antenv.axon_hooks"] = mod
    antenv.axon_hooks = mod
    mod.set_axon_ntff_profile_hook(
        _ntff_profile_via_ctypes("/opt/axon/libaxon_pjrt.so")
    )
    bu.upload_artifacts = lambda tmpdir: tmpdir


def run_traced(inputs):
    _install_ntff_hook()
    out, res = _run(inputs, trace=True)
    return out, res.exec_time_ns


# revision 11
# speedup vs baseline: 1.5731x; 1.5731x over previous
"""Single-head attention (B=4, N=4096, H=768, D=64) on 8 TRN2 NeuronCores.

Sharding: core = (batch, query-half). Each core receives the full batch's
x rows (rotated so its 2048 query rows come first -- softmax over keys is
permutation invariant), computes K/V for all 4096 keys and attention for
its 2048 queries. Output [2048, 64] per core, reassembled on the host.

Matmuls run in float32r (full-rate fp32 streaming, ~1.5e-4 rel err).
The scores contraction (d=64) is padded to K=128 with zero rows: fused
f32r matmuls with K<128 cannot overlap their weight load and run ~1.7x
slower. Softmax denominators come for free from a ones-row appended to
V before transposition. exp() is batched over two PSUM banks to halve
ScalarE per-op overhead.
"""

import sys

sys.path.insert(0, "/opt/trn_rl_repo")

import numpy as np

import concourse.tile as tile
from concourse import bacc, mybir
from concourse.bass_utils import run_bass_kernel_spmd
from concourse.masks import make_identity

B = 4
N = 4096          # keys per batch
NQ = 2048         # queries per core
H = 768
D = 64
P = 128
HC = H // P       # 6 contraction chunks
NKB = N // P      # 32 key blocks
NQB = NQ // 512   # 4 query col-blocks
NTB = N // 512    # 8 token col-blocks for K/V projections
NCORES = 8

DT = mybir.dt.float32
FDT = mybir.dt.float32r

AF = mybir.ActivationFunctionType


def _attention_head(ctx, tc, out, x, Ws, biases):
    nc = tc.nc
    Wq, Wk, Wv = Ws
    bq, bk, bv = biases

    const = ctx.enter_context(tc.tile_pool(name="const", bufs=1))
    big = ctx.enter_context(tc.tile_pool(name="big", bufs=1))
    xin = ctx.enter_context(tc.tile_pool(name="xin", bufs=3))
    # one shared PSUM pool for everything but the attention accumulators;
    # slots sized to the widest tile (1024 f32 = 2 banks) x 3 bufs = 6 banks
    psX = ctx.enter_context(tc.tile_pool(name="psX", bufs=3, space="PSUM"))
    psO = ctx.enter_context(tc.tile_pool(name="psO", bufs=2, space="PSUM"))
    sbE = ctx.enter_context(tc.tile_pool(name="sbE", bufs=4))
    sbo = ctx.enter_context(tc.tile_pool(name="sbo", bufs=4))

    # --- constants -----------------------------------------------------
    ident = const.tile([P, P], DT)
    make_identity(nc, ident)

    w_sb = const.tile([P, 3 * HC * D], FDT)  # [128, 1152]: q,k,v chunks
    for wi, w in enumerate((Wq, Wk, Wv)):
        for c in range(HC):
            nc.sync.dma_start(
                w_sb[:, (wi * HC + c) * D : (wi * HC + c + 1) * D],
                w[c * P : (c + 1) * P, :].bitcast(FDT),
            )
    bias_sb = const.tile([D, 3], DT)
    for bi, b in enumerate((bq, bk, bv)):
        nc.sync.dma_start(
            bias_sb[:, bi : bi + 1], b.rearrange("(p f) -> p f", f=1)
        )

    # --- phase A: xT = x.T into SBUF, [128, HC*4096] ------------------
    xT = big.tile([P, HC * N], FDT)
    for nb in range(NKB):
        xt = xin.tile([P, H], DT)
        nc.sync.dma_start(xt[:], x[nb * P : (nb + 1) * P, :])
        for c in range(HC):
            pt = psX.tile([P, P], DT, tag="x")
            nc.tensor.transpose(pt[:], xt[:, c * P : (c + 1) * P], ident[:])
            nc.vector.tensor_copy(
                xT[:, c * N + nb * P : c * N + (nb + 1) * P], pt[:]
            )

    # --- phase B: projections -----------------------------------------
    # kT/qT are [128, .] with zeroed bottom halves so the scores matmul
    # contracts over K=128 (K=64 would serialize the PE weight load).
    kT = big.tile([P, N], FDT)
    qT = big.tile([P, NQ], FDT)
    vTa = big.tile([D + 1, N], DT)
    nc.gpsimd.memset(kT[D:P, :].bitcast(DT), 0.0)
    nc.gpsimd.memset(qT[D:P, :].bitcast(DT), 0.0)

    def project(dst, wi, tb, bias_col):
        ps = psX.tile([D, 512], DT, tag="x")
        for c in range(HC):
            nc.tensor.matmul(
                ps[:],
                w_sb[:, (wi * HC + c) * D : (wi * HC + c + 1) * D],
                xT[:, c * N + tb * 512 : c * N + tb * 512 + 512],
                start=(c == 0),
                stop=(c == HC - 1),
            )
        nc.scalar.activation(
            dst[0:D, tb * 512 : (tb + 1) * 512],
            ps[:],
            AF.Identity,
            bias=bias_sb[:, bias_col : bias_col + 1],
        )

    for tb in range(NTB):
        project(kT, 1, tb, 1)
    for tb in range(NTB):
        project(vTa, 2, tb, 2)
    for tb in range(NQB):
        project(qT, 0, tb, 0)
    nc.gpsimd.memset(vTa[D : D + 1, :], 1.0)

    # --- phase D: V' blocks [128, 65] per key block -------------------
    v_sb = big.tile([P, NKB * (D + 1)], FDT)
    for kb in range(NKB):
        pt = psX.tile([P, D + 1], DT, tag="x")
        nc.tensor.transpose(
            pt[:],
            vTa[:, kb * P : (kb + 1) * P],
            ident[: D + 1, : D + 1],
        )
        nc.vector.tensor_copy(v_sb[:, kb * (D + 1) : (kb + 1) * (D + 1)], pt[:])

    # --- phase E: attention -------------------------------------------
    # qb pairs kept outer so only two PSUM accumulator banks are live;
    # att matmuls lag the scores/exp by two key blocks so the PE never
    # stalls on the ScalarE exp.
    scale = float(D) ** -0.5
    for qp in range(NQB // 2):
        q0 = 2 * qp * 512
        att = [
            psO.tile([D + 1, 512], DT, name=f"att{qp}_{i}", tag="att")
            for i in range(2)
        ]
        exs = {}
        for kb in range(NKB):
            sc = psX.tile([P, 1024], DT, tag="x")
            for i in range(2):
                nc.tensor.matmul(
                    sc[:, i * 512 : (i + 1) * 512],
                    kT[:, kb * P : (kb + 1) * P],
                    qT[:, q0 + i * 512 : q0 + (i + 1) * 512],
                    start=True,
                    stop=True,
                )
            ex = sbE.tile([P, 1024], FDT, name=f"ex{qp}_{kb}", tag="ex")
            nc.scalar.activation(ex[:], sc[:], AF.Exp, scale=scale)
            exs[kb] = ex
            if kb >= 2:
                _att_mms(nc, att, v_sb, exs.pop(kb - 2), kb - 2)
        for kb in (NKB - 2, NKB - 1):
            _att_mms(nc, att, v_sb, exs.pop(kb), kb)

        # --- normalize + output for this qb pair ----------------------
        for i in range(2):
            asb = sbo.tile([D + 1, 512], DT, tag="asb")
            nc.vector.tensor_copy(asb[:], att[i][:])
            for sub in range(4):
                pt = psX.tile([P, D + 1], DT, tag="x")
                nc.tensor.transpose(
                    pt[:],
                    asb[:, sub * P : (sub + 1) * P],
                    ident[: D + 1, : D + 1],
                )
                rc = sbo.tile([P, 1], DT, tag="rc")
                nc.vector.reciprocal(rc[:], pt[:, D : D + 1])
                ob = sbo.tile([P, D], DT, tag="ob")
                nc.vector.tensor_scalar_mul(ob[:], pt[:, 0:D], rc[:])
                r0 = q0 + i * 512 + sub * P
                nc.sync.dma_start(out[r0 : r0 + P, :], ob[:])


def _att_mms(nc, att, v_sb, ex, kb):
    for i in range(2):
        nc.tensor.matmul(
            att[i][:],
            v_sb[:, kb * (D + 1) : (kb + 1) * (D + 1)],
            ex[:, i * 512 : (i + 1) * 512],
            start=(kb == 0),
            stop=(kb == NKB - 1),
        )


_NC_CACHE = None


def _build():
    global _NC_CACHE
    if _NC_CACHE is not None:
        return _NC_CACHE
    nc = bacc.Bacc(
        "TRN2",
        target_bir_lowering=False,
        debug=False,
        enable_asserts=True,
        num_devices=NCORES,
    )
    x = nc.dram_tensor("x", [N, H], DT, kind="ExternalInput").ap()
    Wq = nc.dram_tensor("Wq", [H, D], DT, kind="ExternalInput").ap()
    bq = nc.dram_tensor("bq", [D], DT, kind="ExternalInput").ap()
    Wk = nc.dram_tensor("Wk", [H, D], DT, kind="ExternalInput").ap()
    bk = nc.dram_tensor("bk", [D], DT, kind="ExternalInput").ap()
    Wv = nc.dram_tensor("Wv", [H, D], DT, kind="ExternalInput").ap()
    bv = nc.dram_tensor("bv", [D], DT, kind="ExternalInput").ap()
    out = nc.dram_tensor("out", [NQ, D], DT, kind="ExternalOutput").ap()

    from contextlib import ExitStack

    with tile.TileContext(nc) as tc:
        with ExitStack() as ctx:
            _attention_head(ctx, tc, out, x, (Wq, Wk, Wv), (bq, bk, bv))
    nc.compile()
    _NC_CACHE = nc
    return nc


def _make_in_maps(inputs):
    x = np.ascontiguousarray(np.asarray(inputs["x"], dtype=np.float32))
    small = {
        k: np.ascontiguousarray(np.asarray(inputs[k], dtype=np.float32))
        for k in ("Wq", "bq", "Wk", "bk", "Wv", "bv")
    }
    in_maps = []
    for core in range(NCORES):
        b, h = divmod(core, 2)
        xb = x[b]
        if h == 1:
            xb = np.concatenate([xb[NQ:], xb[:NQ]], axis=0)
        in_maps.append({"x": np.ascontiguousarray(xb), **small})
    return in_maps


def _run(inputs, trace=False):
    nc = _build()
    res = run_bass_kernel_spmd(
        nc, _make_in_maps(inputs), core_ids=list(range(NCORES)), trace=trace
    )
    out = np.empty((B, N, D), dtype=np.float32)
    for core in range(NCORES):
        b, h = divmod(core, 2)
        out[b, h * NQ : (h + 1) * NQ] = res.results[core]["out"]
    return out, res


def kernel(**inputs):
    out, _ = _run(inputs, trace=False)
    return out


def _install_ntff_hook():
    """Register the axon NTFF profiling hook that this image's antenv lacks."""
    import types

    try:
        import antenv.axon_hooks  # noqa: F401

        return
    except ImportError:
        pass
    import antenv
    from trn_agent_boot.trn_boot import _ntff_profile_via_ctypes

    import concourse.bass_utils as bu

    mod = types.ModuleType("antenv.axon_hooks")
    _h = [None]
    mod.set_axon_ntff_profile_hook = lambda h: _h.__setitem__(0, h)
    mod.get_axon_ntff_profile_hook = lambda: _h[0]
    sys.modules["antenv.axon_hooks"] = mod
    antenv.axon_hooks = mod
    mod.set_axon_ntff_profile_hook(
        _ntff_profile_via_ctypes("/opt/axon/libaxon_pjrt.so")
    )
    bu.upload_artifacts = lambda tmpdir: tmpdir


def run_traced(inputs):
    _install_ntff_hook()
    out, res = _run(inputs, trace=True)
    return out, res.exec_time_ns


# revision 15
# speedup vs baseline: 1.7282x; 1.0986x over previous
"""Single-head attention (B=4, N=4096, H=768, D=64) on 8 TRN2 NeuronCores.

Sharding: core = (batch b, sequence half h). Each core receives only its
own 2048 rows of x[b], projects Q/K/V for them, then all-gathers the
projected K and V' blocks within the (b,0)/(b,1) pair -- 1 MB of
activations instead of re-loading and re-projecting the partner's 6 MB
of x. Softmax over keys is permutation invariant, so both cores use the
same gathered key order. Output [2048, 64] per core, reassembled host-side.

Matmuls run in float32r (full-rate fp32 streaming, ~1.5e-4 rel err).
The scores contraction (d=64) is padded to K=128 with zero rows: fused
f32r matmuls with K<128 cannot overlap their weight load and run ~1.7x
slower. K and V share one projection matmul (stationary [Wk|Wv], M=128).
Softmax denominators come free from a ones column in the V' blocks.
exp() is batched over two PSUM banks to halve ScalarE per-op overhead.
"""

import sys

sys.path.insert(0, "/opt/trn_rl_repo")

import numpy as np

import concourse.tile as tile
from concourse import bacc, mybir
from concourse.bass_utils import run_bass_kernel_spmd
from concourse.masks import make_identity

B = 4
N = 4096          # keys per batch
NQ = 2048         # queries / own keys per core
H = 768
D = 64
P = 128
HC = H // P       # 6 contraction chunks
NKB = N // P      # 32 key blocks (post-gather)
NOB = NQ // P     # 16 own key blocks
NTB = NQ // 512   # 4 token col-blocks for projections
NCORES = 8

DT = mybir.dt.float32
FDT = mybir.dt.float32r

AF = mybir.ActivationFunctionType


def _attention_head(ctx, tc, out, x, Ws, biases):
    nc = tc.nc
    Wq, Wk, Wv = Ws
    bq, bk, bv = biases

    const = ctx.enter_context(tc.tile_pool(name="const", bufs=1))
    big = ctx.enter_context(tc.tile_pool(name="big", bufs=1))
    xin = ctx.enter_context(tc.tile_pool(name="xin", bufs=3))
    psX = ctx.enter_context(tc.tile_pool(name="psX", bufs=3, space="PSUM"))
    psO = ctx.enter_context(tc.tile_pool(name="psO", bufs=2, space="PSUM"))
    sbE = ctx.enter_context(tc.tile_pool(name="sbE", bufs=4))
    sbo = ctx.enter_context(tc.tile_pool(name="sbo", bufs=4))
    dram = ctx.enter_context(tc.tile_pool(name="dram", bufs=1, space="DRAM"))

    # --- constants (SWDGE queue; keeps the sync queue free for x) ------
    ident = const.tile([P, P], DT)
    make_identity(nc, ident)

    w_q = const.tile([P, HC * D], FDT)
    nc.gpsimd.dma_start(
        w_q[:].rearrange("p (c d) -> p c d", d=D),
        Wq.rearrange("(c p) d -> p c d", p=P).bitcast(FDT),
    )
    w_kv = const.tile([P, HC * P], FDT)  # chunk c: [Wk_c | Wv_c]
    for wi, w in enumerate((Wk, Wv)):
        nc.gpsimd.dma_start(
            w_kv[:].rearrange("p (c g) -> p c g", g=P)[:, :, wi * D : (wi + 1) * D],
            w.rearrange("(c p) d -> p c d", p=P).bitcast(FDT),
        )
    bias_sb = const.tile([P, 3], DT)
    for bi, b, r0 in ((0, bq, 0), (1, bk, 0), (2, bv, D)):
        nc.gpsimd.dma_start(
            bias_sb[r0 : r0 + D, bi : bi + 1], b.rearrange("(p f) -> p f", f=1)
        )

    # --- phase A: xT = x_own.T into SBUF, [128, HC*2048] ---------------
    xT = big.tile([P, HC * NQ], FDT)
    for nb in range(NOB):
        xt = xin.tile([P, H], DT)
        nc.sync.dma_start(xt[:], x[nb * P : (nb + 1) * P, :])
        for c in range(HC):
            pt = psX.tile([P, P], DT, tag="x")
            nc.tensor.transpose(pt[:], xt[:, c * P : (c + 1) * P], ident[:])
            nc.vector.tensor_copy(
                xT[:, c * NQ + nb * P : c * NQ + (nb + 1) * P], pt[:]
            )

    # --- phase B: projections over own tokens --------------------------
    # kv_sb rows 0:64 = K^T + bk, rows 64:128 = V^T + bv (both [64, 2048])
    kv_sb = big.tile([P, NQ], DT)
    qT = big.tile([P, NQ], FDT)
    nc.gpsimd.memset(qT[D:P, :].bitcast(DT), 0.0)

    for tb in range(NTB):
        s = slice(tb * 512, (tb + 1) * 512)
        ps = psX.tile([P, 512], DT, tag="x", name="pkv")
        for c in range(HC):
            nc.tensor.matmul(
                ps[:],
                w_kv[:, c * P : (c + 1) * P],
                xT[:, c * NQ + tb * 512 : c * NQ + tb * 512 + 512],
                start=(c == 0),
                stop=(c == HC - 1),
            )
        nc.scalar.activation(
            kv_sb[0:D, s], ps[0:D, :], AF.Identity, bias=bias_sb[0:D, 1:2]
        )
        nc.scalar.activation(
            kv_sb[D:P, s], ps[D:P, :], AF.Identity, bias=bias_sb[D:P, 2:3]
        )

    # --- all-gather #1: K^T halves within the core pair -----------------
    GROUPS = [[2 * i, 2 * i + 1] for i in range(NCORES // 2)]
    kt_in = dram.tile([D, NQ], DT)
    kt_out = dram.tile([2 * D, NQ], DT)
    nc.sync.dma_start(kt_in[:], kv_sb[0:D, :])
    nc.gpsimd.collective_compute(
        "AllGather",
        mybir.AluOpType.bypass,
        replica_groups=GROUPS,
        ins=[kt_in.opt()],
        outs=[kt_out.opt()],
    )

    # --- phase D: own V' blocks [128, 65] (col 64 = ones) ---------------
    v_own = big.tile([P, NOB * (D + 1)], DT)
    nc.gpsimd.memset(
        v_own[:].rearrange("p (k c) -> p k c", c=D + 1)[:, :, D : D + 1], 1.0
    )
    for kb in range(NOB):
        pt = psX.tile([P, D], DT, tag="x", name="pv")
        nc.tensor.transpose(
            pt[:], kv_sb[D:P, kb * P : (kb + 1) * P], ident[D:P, D:P]
        )
        nc.vector.tensor_copy(v_own[:, kb * (D + 1) : kb * (D + 1) + D], pt[:])

    # --- all-gather #2: V' halves ---------------------------------------
    v_in = dram.tile([P, NOB * (D + 1)], DT)
    v_out = dram.tile([2 * P, NOB * (D + 1)], DT)
    nc.sync.dma_start(v_in[:], v_own[:])
    nc.gpsimd.collective_compute(
        "AllGather",
        mybir.AluOpType.bypass,
        replica_groups=GROUPS,
        ins=[v_in.opt()],
        outs=[v_out.opt()],
    )

    # --- phase B2: Q projection (overlaps the collectives) --------------
    for tb in range(NTB):
        ps = psX.tile([D, 512], DT, tag="x", name="pq")
        for c in range(HC):
            nc.tensor.matmul(
                ps[:],
                w_q[:, c * D : (c + 1) * D],
                xT[:, c * NQ + tb * 512 : c * NQ + tb * 512 + 512],
                start=(c == 0),
                stop=(c == HC - 1),
            )
        nc.scalar.activation(
            qT[0:D, tb * 512 : (tb + 1) * 512],
            ps[:],
            AF.Identity,
            bias=bias_sb[0:D, 0:1],
        )

    # --- unpack gathered K/V' -------------------------------------------
    kT = big.tile([P, N], FDT)
    nc.gpsimd.memset(kT[D:P, :].bitcast(DT), 0.0)
    v_sb = big.tile([P, NKB * (D + 1)], FDT)
    for h in range(2):
        nc.sync.dma_start(
            kT[0:D, h * NQ : (h + 1) * NQ],
            kt_out[h * D : (h + 1) * D, :].bitcast(FDT),
        )
        nc.sync.dma_start(
            v_sb[:, h * NOB * (D + 1) : (h + 1) * NOB * (D + 1)],
            v_out[h * P : (h + 1) * P, :].bitcast(FDT),
        )

    # --- phase E: attention ---------------------------------------------
    scale = float(D) ** -0.5
    for qp in range(2):
        q0 = qp * 1024
        att = [
            psO.tile([D + 1, 512], DT, name=f"att{qp}_{i}", tag="att")
            for i in range(2)
        ]
        exs = {}
        for kb in range(NKB):
            sc = psX.tile([P, 1024], DT, tag="x", name="sc")
            for i in range(2):
                nc.tensor.matmul(
                    sc[:, i * 512 : (i + 1) * 512],
                    kT[:, kb * P : (kb + 1) * P],
                    qT[:, q0 + i * 512 : q0 + (i + 1) * 512],
                    start=True,
                    stop=True,
                )
            ex = sbE.tile([P, 1024], FDT, name=f"ex{qp}_{kb}", tag="ex")
            nc.scalar.activation(ex[:], sc[:], AF.Exp, scale=scale)
            exs[kb] = ex
            if kb >= 2:
                _att_mms(nc, att, v_sb, exs.pop(kb - 2), kb - 2)
        for kb in (NKB - 2, NKB - 1):
            _att_mms(nc, att, v_sb, exs.pop(kb), kb)

        # --- normalize + batched output for this qb pair -----------------
        ob = sbo.tile([P, 8 * D], DT, tag="ob")
        for i in range(2):
            asb = sbo.tile([D + 1, 512], DT, tag="asb")
            nc.vector.tensor_copy(asb[:], att[i][:])
            for sub in range(4):
                pt = psX.tile([P, D + 1], DT, tag="x", name="pf")
                nc.tensor.transpose(
                    pt[:],
                    asb[:, sub * P : (sub + 1) * P],
                    ident[: D + 1, : D + 1],
                )
                rc = sbo.tile([P, 1], DT, tag="rc")
                nc.vector.reciprocal(rc[:], pt[:, D : D + 1])
                j = i * 4 + sub
                nc.vector.tensor_scalar_mul(
                    ob[:, j * D : (j + 1) * D], pt[:, 0:D], rc[:]
                )
        nc.sync.dma_start(
            out[q0 : q0 + 1024, :].rearrange("(s p) d -> p s d", p=P),
            ob[:].rearrange("p (s d) -> p s d", d=D),
        )


def _att_mms(nc, att, v_sb, ex, kb):
    for i in range(2):
        nc.tensor.matmul(
            att[i][:],
            v_sb[:, kb * (D + 1) : (kb + 1) * (D + 1)],
            ex[:, i * 512 : (i + 1) * 512],
            start=(kb == 0),
            stop=(kb == NKB - 1),
        )


_NC_CACHE = None


def _build():
    global _NC_CACHE
    if _NC_CACHE is not None:
        return _NC_CACHE
    nc = bacc.Bacc(
        "TRN2",
        target_bir_lowering=False,
        debug=False,
        enable_asserts=True,
        num_devices=NCORES,
    )
    x = nc.dram_tensor("x", [NQ, H], DT, kind="ExternalInput").ap()
    Wq = nc.dram_tensor("Wq", [H, D], DT, kind="ExternalInput").ap()
    bq = nc.dram_tensor("bq", [D], DT, kind="ExternalInput").ap()
    Wk = nc.dram_tensor("Wk", [H, D], DT, kind="ExternalInput").ap()
    bk = nc.dram_tensor("bk", [D], DT, kind="ExternalInput").ap()
    Wv = nc.dram_tensor("Wv", [H, D], DT, kind="ExternalInput").ap()
    bv = nc.dram_tensor("bv", [D], DT, kind="ExternalInput").ap()
    out = nc.dram_tensor("out", [NQ, D], DT, kind="ExternalOutput").ap()

    from contextlib import ExitStack

    with tile.TileContext(nc) as tc:
        with ExitStack() as ctx:
            _attention_head(ctx, tc, out, x, (Wq, Wk, Wv), (bq, bk, bv))
    nc.compile()
    _NC_CACHE = nc
    return nc


def _make_in_maps(inputs):
    x = np.ascontiguousarray(np.asarray(inputs["x"], dtype=np.float32))
    small = {
        k: np.ascontiguousarray(np.asarray(inputs[k], dtype=np.float32))
        for k in ("Wq", "bq", "Wk", "bk", "Wv", "bv")
    }
    in_maps = []
    for core in range(NCORES):
        b, h = divmod(core, 2)
        xb = np.ascontiguousarray(x[b, h * NQ : (h + 1) * NQ])
        in_maps.append({"x": xb, **small})
    return in_maps


def _run(inputs, trace=False):
    nc = _build()
    res = run_bass_kernel_spmd(
        nc, _make_in_maps(inputs), core_ids=list(range(NCORES)), trace=trace
    )
    out = np.empty((B, N, D), dtype=np.float32)
    for core in range(NCORES):
        b, h = divmod(core, 2)
        out[b, h * NQ : (h + 1) * NQ] = res.results[core]["out"]
    return out, res


def kernel(**inputs):
    out, _ = _run(inputs, trace=False)
    return out


def _install_ntff_hook():
    """Register the axon NTFF profiling hook that this image's antenv lacks."""
    import types

    try:
        import antenv.axon_hooks  # noqa: F401

        return
    except ImportError:
        pass
    import antenv
    from trn_agent_boot.trn_boot import _ntff_profile_via_ctypes

    import concourse.bass_utils as bu

    mod = types.ModuleType("antenv.axon_hooks")
    _h = [None]
    mod.set_axon_ntff_profile_hook = lambda h: _h.__setitem__(0, h)
    mod.get_axon_ntff_profile_hook = lambda: _h[0]
    sys.modules["antenv.axon_hooks"] = mod
    antenv.axon_hooks = mod
    mod.set_axon_ntff_profile_hook(
        _ntff_profile_via_ctypes("/opt/axon/libaxon_pjrt.so")
    )
    bu.upload_artifacts = lambda tmpdir: tmpdir


def run_traced(inputs):
    _install_ntff_hook()
    out, res = _run(inputs, trace=True)
    return out, res.exec_time_ns


# revision 16
# speedup vs baseline: 1.7382x; 1.0058x over previous
"""Single-head attention (B=4, N=4096, H=768, D=64) on 8 TRN2 NeuronCores.

Sharding: core = (batch b, sequence half h). Each core receives only its
own 2048 rows of x[b], projects Q/K/V for them, then all-gathers the
projected K and V' blocks within the (b,0)/(b,1) pair -- 1 MB of
activations instead of re-loading and re-projecting the partner's 6 MB
of x. Softmax over keys is permutation invariant, so both cores use the
same gathered key order. Output [2048, 64] per core, reassembled host-side.

Matmuls run in float32r (full-rate fp32 streaming, ~1.5e-4 rel err).
The scores contraction (d=64) is padded to K=128 with zero rows: fused
f32r matmuls with K<128 cannot overlap their weight load and run ~1.7x
slower. K and V share one projection matmul (stationary [Wk|Wv], M=128).
Softmax denominators come free from a ones column in the V' blocks.
exp() is batched over two PSUM banks to halve ScalarE per-op overhead.
"""

import sys

sys.path.insert(0, "/opt/trn_rl_repo")

import numpy as np

import concourse.tile as tile
from concourse import bacc, mybir
from concourse.bass_utils import run_bass_kernel_spmd
from concourse.masks import make_identity

B = 4
N = 4096          # keys per batch
NQ = 2048         # queries / own keys per core
H = 768
D = 64
P = 128
HC = H // P       # 6 contraction chunks
NKB = N // P      # 32 key blocks (post-gather)
NOB = NQ // P     # 16 own key blocks
NTB = NQ // 512   # 4 token col-blocks for projections
NCORES = 8

DT = mybir.dt.float32
FDT = mybir.dt.float32r

AF = mybir.ActivationFunctionType


def _attention_head(ctx, tc, out, x, Ws, biases):
    nc = tc.nc
    Wq, Wk, Wv = Ws
    bq, bk, bv = biases

    const = ctx.enter_context(tc.tile_pool(name="const", bufs=1))
    big = ctx.enter_context(tc.tile_pool(name="big", bufs=1))
    xin = ctx.enter_context(tc.tile_pool(name="xin", bufs=3))
    psX = ctx.enter_context(tc.tile_pool(name="psX", bufs=3, space="PSUM"))
    psO = ctx.enter_context(tc.tile_pool(name="psO", bufs=2, space="PSUM"))
    sbE = ctx.enter_context(tc.tile_pool(name="sbE", bufs=4))
    sbo = ctx.enter_context(tc.tile_pool(name="sbo", bufs=4))
    dram = ctx.enter_context(tc.tile_pool(name="dram", bufs=1, space="DRAM"))

    # --- constants (SWDGE queue; keeps the sync queue free for x) ------
    ident = const.tile([P, P], DT)
    make_identity(nc, ident)

    w_q = const.tile([P, HC * D], FDT)
    nc.gpsimd.dma_start(
        w_q[:].rearrange("p (c d) -> p c d", d=D),
        Wq.rearrange("(c p) d -> p c d", p=P).bitcast(FDT),
    )
    w_kv = const.tile([P, HC * P], FDT)  # chunk c: [Wk_c | Wv_c]
    for wi, w in enumerate((Wk, Wv)):
        nc.gpsimd.dma_start(
            w_kv[:].rearrange("p (c g) -> p c g", g=P)[:, :, wi * D : (wi + 1) * D],
            w.rearrange("(c p) d -> p c d", p=P).bitcast(FDT),
        )
    bias_sb = const.tile([P, 3], DT)
    for bi, b, r0 in ((0, bq, 0), (1, bk, 0), (2, bv, D)):
        nc.gpsimd.dma_start(
            bias_sb[r0 : r0 + D, bi : bi + 1], b.rearrange("(p f) -> p f", f=1)
        )

    # --- phase A: xT = x_own.T into SBUF, [128, HC*2048] ---------------
    xT = big.tile([P, HC * NQ], FDT)
    for nb in range(NOB):
        xt = xin.tile([P, H], DT)
        nc.sync.dma_start(xt[:], x[nb * P : (nb + 1) * P, :])
        for c in range(HC):
            pt = psX.tile([P, P], DT, tag="x")
            nc.tensor.transpose(pt[:], xt[:, c * P : (c + 1) * P], ident[:])
            nc.vector.tensor_copy(
                xT[:, c * NQ + nb * P : c * NQ + (nb + 1) * P], pt[:]
            )

    # --- phase B: projections over own tokens --------------------------
    # kv_sb rows 0:64 = K^T + bk, rows 64:128 = V^T + bv (both [64, 2048])
    kv_sb = big.tile([P, NQ], DT)
    qT = big.tile([P, NQ], FDT)
    nc.gpsimd.memset(qT[D:P, :].bitcast(DT), 0.0)

    for tb in range(NTB):
        s = slice(tb * 512, (tb + 1) * 512)
        ps = psX.tile([P, 512], DT, tag="x", name="pkv")
        for c in range(HC):
            nc.tensor.matmul(
                ps[:],
                w_kv[:, c * P : (c + 1) * P],
                xT[:, c * NQ + tb * 512 : c * NQ + tb * 512 + 512],
                start=(c == 0),
                stop=(c == HC - 1),
            )
        nc.scalar.activation(
            kv_sb[0:D, s], ps[0:D, :], AF.Identity, bias=bias_sb[0:D, 1:2]
        )
        nc.scalar.activation(
            kv_sb[D:P, s], ps[D:P, :], AF.Identity, bias=bias_sb[D:P, 2:3]
        )

    # --- all-gather: packed K^T/V^T halves within the core pair ---------
    GROUPS = [[2 * i, 2 * i + 1] for i in range(NCORES // 2)]
    kv_in = dram.tile([P, NQ], DT)
    kv_out = dram.tile([2 * P, NQ], DT)
    nc.sync.dma_start(kv_in[:], kv_sb[:])
    nc.gpsimd.collective_compute(
        "AllGather",
        mybir.AluOpType.bypass,
        replica_groups=GROUPS,
        ins=[kv_in.opt()],
        outs=[kv_out.opt()],
    )

    # --- phase B2: Q projection (overlaps the collective) ---------------
    for tb in range(NTB):
        ps = psX.tile([D, 512], DT, tag="x", name="pq")
        for c in range(HC):
            nc.tensor.matmul(
                ps[:],
                w_q[:, c * D : (c + 1) * D],
                xT[:, c * NQ + tb * 512 : c * NQ + tb * 512 + 512],
                start=(c == 0),
                stop=(c == HC - 1),
            )
        nc.scalar.activation(
            qT[0:D, tb * 512 : (tb + 1) * 512],
            ps[:],
            AF.Identity,
            bias=bias_sb[0:D, 0:1],
        )

    # --- unpack gathered K^T/V^T ----------------------------------------
    kT = big.tile([P, N], FDT)
    nc.gpsimd.memset(kT[D:P, :].bitcast(DT), 0.0)
    vt_sb = big.tile([D, N], DT)
    for h in range(2):
        nc.sync.dma_start(
            kT[0:D, h * NQ : (h + 1) * NQ],
            kv_out[h * P : h * P + D, :].bitcast(FDT),
        )
    for h in range(2):
        nc.sync.dma_start(
            vt_sb[:, h * NQ : (h + 1) * NQ], kv_out[h * P + D : (h + 1) * P, :]
        )

    # --- phase D: V' blocks [128, 65] (col 64 = ones), post-gather ------
    v_sb = big.tile([P, NKB * (D + 1)], FDT)
    nc.gpsimd.memset(
        v_sb[:].bitcast(DT).rearrange("p (k c) -> p k c", c=D + 1)[
            :, :, D : D + 1
        ],
        1.0,
    )
    for kb in range(NKB):
        pt = psX.tile([P, D], DT, tag="x", name="pv")
        nc.tensor.transpose(
            pt[:], vt_sb[:, kb * P : (kb + 1) * P], ident[:D, :D]
        )
        nc.vector.tensor_copy(v_sb[:, kb * (D + 1) : kb * (D + 1) + D], pt[:])

    # --- phase E: attention ---------------------------------------------
    scale = float(D) ** -0.5
    for qp in range(2):
        q0 = qp * 1024
        att = [
            psO.tile([D + 1, 512], DT, name=f"att{qp}_{i}", tag="att")
            for i in range(2)
        ]
        exs = {}
        for kb in range(NKB):
            sc = psX.tile([P, 1024], DT, tag="x", name="sc")
            for i in range(2):
                nc.tensor.matmul(
                    sc[:, i * 512 : (i + 1) * 512],
                    kT[:, kb * P : (kb + 1) * P],
                    qT[:, q0 + i * 512 : q0 + (i + 1) * 512],
                    start=True,
                    stop=True,
                )
            ex = sbE.tile([P, 1024], FDT, name=f"ex{qp}_{kb}", tag="ex")
            nc.scalar.activation(ex[:], sc[:], AF.Exp, scale=scale)
            exs[kb] = ex
            if kb >= 2:
                _att_mms(nc, att, v_sb, exs.pop(kb - 2), kb - 2)
        for kb in (NKB - 2, NKB - 1):
            _att_mms(nc, att, v_sb, exs.pop(kb), kb)

        # --- normalize + batched output for this qb pair -----------------
        ob = sbo.tile([P, 8 * D], DT, tag="ob")
        for i in range(2):
            asb = sbo.tile([D + 1, 512], DT, tag="asb")
            nc.vector.tensor_copy(asb[:], att[i][:])
            for sub in range(4):
                pt = psX.tile([P, D + 1], DT, tag="x", name="pf")
                nc.tensor.transpose(
                    pt[:],
                    asb[:, sub * P : (sub + 1) * P],
                    ident[: D + 1, : D + 1],
                )
                rc = sbo.tile([P, 1], DT, tag="rc")
                nc.vector.reciprocal(rc[:], pt[:, D : D + 1])
                j = i * 4 + sub
                nc.vector.tensor_scalar_mul(
                    ob[:, j * D : (j + 1) * D], pt[:, 0:D], rc[:]
                )
        nc.sync.dma_start(
            out[q0 : q0 + 1024, :].rearrange("(s p) d -> p s d", p=P),
            ob[:].rearrange("p (s d) -> p s d", d=D),
        )


def _att_mms(nc, att, v_sb, ex, kb):
    for i in range(2):
        nc.tensor.matmul(
            att[i][:],
            v_sb[:, kb * (D + 1) : (kb + 1) * (D + 1)],
            ex[:, i * 512 : (i + 1) * 512],
            start=(kb == 0),
            stop=(kb == NKB - 1),
        )


_NC_CACHE = None


def _build():
    global _NC_CACHE
    if _NC_CACHE is not None:
        return _NC_CACHE
    nc = bacc.Bacc(
        "TRN2",
        target_bir_lowering=False,
        debug=False,
        enable_asserts=True,
        num_devices=NCORES,
    )
    x = nc.dram_tensor("x", [NQ, H], DT, kind="ExternalInput").ap()
    Wq = nc.dram_tensor("Wq", [H, D], DT, kind="ExternalInput").ap()
    bq = nc.dram_tensor("bq", [D], DT, kind="ExternalInput").ap()
    Wk = nc.dram_tensor("Wk", [H, D], DT, kind="ExternalInput").ap()
    bk = nc.dram_tensor("bk", [D], DT, kind="ExternalInput").ap()
    Wv = nc.dram_tensor("Wv", [H, D], DT, kind="ExternalInput").ap()
    bv = nc.dram_tensor("bv", [D], DT, kind="ExternalInput").ap()
    out = nc.dram_tensor("out", [NQ, D], DT, kind="ExternalOutput").ap()

    from contextlib import ExitStack

    with tile.TileContext(nc) as tc:
        with ExitStack() as ctx:
            _attention_head(ctx, tc, out, x, (Wq, Wk, Wv), (bq, bk, bv))
    nc.compile()
    _NC_CACHE = nc
    return nc


def _make_in_maps(inputs):
    x = np.ascontiguousarray(np.asarray(inputs["x"], dtype=np.float32))
    small = {
        k: np.ascontiguousarray(np.asarray(inputs[k], dtype=np.float32))
        for k in ("Wq", "bq", "Wk", "bk", "Wv", "bv")
    }
    in_maps = []
    for core in range(NCORES):
        b, h = divmod(core, 2)
        xb = np.ascontiguousarray(x[b, h * NQ : (h + 1) * NQ])
        in_maps.append({"x": xb, **small})
    return in_maps


def _run(inputs, trace=False):
    nc = _build()
    res = run_bass_kernel_spmd(
        nc, _make_in_maps(inputs), core_ids=list(range(NCORES)), trace=trace
    )
    out = np.empty((B, N, D), dtype=np.float32)
    for core in range(NCORES):
        b, h = divmod(core, 2)
        out[b, h * NQ : (h + 1) * NQ] = res.results[core]["out"]
    return out, res


def kernel(**inputs):
    out, _ = _run(inputs, trace=False)
    return out


def _install_ntff_hook():
    """Register the axon NTFF profiling hook that this image's antenv lacks."""
    import types

    try:
        import antenv.axon_hooks  # noqa: F401

        return
    except ImportError:
        pass
    import antenv
    from trn_agent_boot.trn_boot import _ntff_profile_via_ctypes

    import concourse.bass_utils as bu

    mod = types.ModuleType("antenv.axon_hooks")
    _h = [None]
    mod.set_axon_ntff_profile_hook = lambda h: _h.__setitem__(0, h)
    mod.get_axon_ntff_profile_hook = lambda: _h[0]
    sys.modules["antenv.axon_hooks"] = mod
    antenv.axon_hooks = mod
    mod.set_axon_ntff_profile_hook(
        _ntff_profile_via_ctypes("/opt/axon/libaxon_pjrt.so")
    )
    bu.upload_artifacts = lambda tmpdir: tmpdir


def run_traced(inputs):
    _install_ntff_hook()
    out, res = _run(inputs, trace=True)
    return out, res.exec_time_ns


# revision 19
# speedup vs baseline: 1.8021x; 1.0368x over previous
"""Single-head attention (B=4, N=4096, H=768, D=64) on 8 TRN2 NeuronCores.

Sharding: core = (batch b, sequence half h). Each core receives only its
own 2048 rows of x[b], projects Q/K/V for them, then all-gathers the
projected K and V' blocks within the (b,0)/(b,1) pair -- 1 MB of
activations instead of re-loading and re-projecting the partner's 6 MB
of x. Softmax over keys is permutation invariant, so both cores use the
same gathered key order. Output [2048, 64] per core, reassembled host-side.

Matmuls run in float32r (full-rate fp32 streaming, ~1.5e-4 rel err).
The scores contraction (d=64) is padded to K=128 with zero rows: fused
f32r matmuls with K<128 cannot overlap their weight load and run ~1.7x
slower. K and V share one projection matmul (stationary [Wk|Wv], M=128).
Softmax denominators come free from a ones column in the V' blocks.
exp() is batched over two PSUM banks to halve ScalarE per-op overhead.
"""

import sys

sys.path.insert(0, "/opt/trn_rl_repo")

import numpy as np

import concourse.tile as tile
from concourse import bacc, mybir
from concourse.bass_utils import run_bass_kernel_spmd
from concourse.masks import make_identity

B = 4
N = 4096          # keys per batch
NQ = 2048         # queries / own keys per core
H = 768
D = 64
P = 128
HC = H // P       # 6 contraction chunks
NKB = N // P      # 32 key blocks (post-gather)
NOB = NQ // P     # 16 own key blocks
NTB = NQ // 512   # 4 token col-blocks for projections
NCORES = 8

DT = mybir.dt.float32
FDT = mybir.dt.float32r

AF = mybir.ActivationFunctionType


def _attention_head(ctx, tc, out, x, Ws, biases):
    nc = tc.nc
    Wq, Wk, Wv = Ws
    bq, bk, bv = biases

    const = ctx.enter_context(tc.tile_pool(name="const", bufs=1))
    big = ctx.enter_context(tc.tile_pool(name="big", bufs=1))
    xin = ctx.enter_context(tc.tile_pool(name="xin", bufs=3))
    psX = ctx.enter_context(tc.tile_pool(name="psX", bufs=3, space="PSUM"))
    psO = ctx.enter_context(tc.tile_pool(name="psO", bufs=2, space="PSUM"))
    sbE = ctx.enter_context(tc.tile_pool(name="sbE", bufs=4))
    sbo = ctx.enter_context(tc.tile_pool(name="sbo", bufs=4))
    dram = ctx.enter_context(tc.tile_pool(name="dram", bufs=1, space="DRAM"))

    # --- constants (SWDGE queue; keeps the sync queue free for x) ------
    ident = const.tile([P, P], DT)
    make_identity(nc, ident)

    # Tiny warm-up gather: pays the collective launch latency and pair
    # start-skew while the PE is busy with transposes.
    GROUPS = [[2 * i, 2 * i + 1] for i in range(NCORES // 2)]
    warm_in = dram.tile([1, 16], DT)
    warm_out = dram.tile([2, 16], DT)
    nc.gpsimd.dma_start(warm_in[:], ident[0:1, 0:16])
    nc.gpsimd.collective_compute(
        "AllGather",
        mybir.AluOpType.bypass,
        replica_groups=GROUPS,
        ins=[warm_in.opt()],
        outs=[warm_out.opt()],
    )

    w_q = const.tile([P, HC * D], FDT)
    nc.gpsimd.dma_start(
        w_q[:].rearrange("p (c d) -> p c d", d=D),
        Wq.rearrange("(c p) d -> p c d", p=P).bitcast(FDT),
    )
    w_kv = const.tile([P, HC * P], FDT)  # chunk c: [Wk_c | Wv_c]
    for wi, w in enumerate((Wk, Wv)):
        nc.gpsimd.dma_start(
            w_kv[:].rearrange("p (c g) -> p c g", g=P)[:, :, wi * D : (wi + 1) * D],
            w.rearrange("(c p) d -> p c d", p=P).bitcast(FDT),
        )
    bias_sb = const.tile([P, 3], DT)
    for bi, b, r0 in ((0, bq, 0), (1, bk, 0), (2, bv, D)):
        nc.gpsimd.dma_start(
            bias_sb[r0 : r0 + D, bi : bi + 1], b.rearrange("(p f) -> p f", f=1)
        )

    # --- phase A: xT = x_own.T into SBUF, [128, HC*2048] ---------------
    xT = big.tile([P, HC * NQ], FDT)
    for nb in range(NOB):
        xt = xin.tile([P, H], DT)
        nc.sync.dma_start(xt[:], x[nb * P : (nb + 1) * P, :])
        for c in range(HC):
            pt = psX.tile([P, P], DT, tag="x")
            nc.tensor.transpose(pt[:], xt[:, c * P : (c + 1) * P], ident[:])
            nc.vector.tensor_copy(
                xT[:, c * NQ + nb * P : c * NQ + (nb + 1) * P], pt[:]
            )

    # --- phase B: projections over own tokens --------------------------
    # kv_sb rows 0:64 = K^T + bk, rows 64:128 = V^T + bv (both [64, 2048])
    kv_sb = big.tile([P, NQ], DT)
    qT = big.tile([P, NQ], FDT)
    nc.gpsimd.memset(qT[D:P, :].bitcast(DT), 0.0)

    for tb in range(NTB):
        s = slice(tb * 512, (tb + 1) * 512)
        ps = psX.tile([P, 512], DT, tag="x", name="pkv")
        for c in range(HC):
            nc.tensor.matmul(
                ps[:],
                w_kv[:, c * P : (c + 1) * P],
                xT[:, c * NQ + tb * 512 : c * NQ + tb * 512 + 512],
                start=(c == 0),
                stop=(c == HC - 1),
            )
        nc.scalar.activation(
            kv_sb[0:D, s], ps[0:D, :], AF.Identity, bias=bias_sb[0:D, 1:2]
        )
        nc.scalar.activation(
            kv_sb[D:P, s], ps[D:P, :], AF.Identity, bias=bias_sb[D:P, 2:3]
        )

    # --- all-gather: packed K^T/V^T halves within the core pair ---------
    kv_in = dram.tile([P, NQ], DT)
    kv_out = dram.tile([2 * P, NQ], DT)
    nc.sync.dma_start(kv_in[:], kv_sb[:])
    nc.gpsimd.collective_compute(
        "AllGather",
        mybir.AluOpType.bypass,
        replica_groups=GROUPS,
        ins=[kv_in.opt()],
        outs=[kv_out.opt()],
    )

    # --- phase B2: Q projection (overlaps the collective) ---------------
    for tb in range(NTB):
        ps = psX.tile([D, 512], DT, tag="x", name="pq")
        for c in range(HC):
            nc.tensor.matmul(
                ps[:],
                w_q[:, c * D : (c + 1) * D],
                xT[:, c * NQ + tb * 512 : c * NQ + tb * 512 + 512],
                start=(c == 0),
                stop=(c == HC - 1),
            )
        nc.scalar.activation(
            qT[0:D, tb * 512 : (tb + 1) * 512],
            ps[:],
            AF.Identity,
            bias=bias_sb[0:D, 0:1],
        )

    # --- unpack gathered K^T/V^T ----------------------------------------
    kT = big.tile([P, N], FDT)
    nc.gpsimd.memset(kT[D:P, :].bitcast(DT), 0.0)
    vt_sb = big.tile([D, N], DT)
    for h in range(2):
        nc.sync.dma_start(
            kT[0:D, h * NQ : (h + 1) * NQ],
            kv_out[h * P : h * P + D, :].bitcast(FDT),
        )
    for h in range(2):
        nc.sync.dma_start(
            vt_sb[:, h * NQ : (h + 1) * NQ], kv_out[h * P + D : (h + 1) * P, :]
        )

    # --- phase D: V' blocks [128, 65] (col 64 = ones), post-gather ------
    v_sb = big.tile([P, NKB * (D + 1)], FDT)
    nc.gpsimd.memset(
        v_sb[:].bitcast(DT).rearrange("p (k c) -> p k c", c=D + 1)[
            :, :, D : D + 1
        ],
        1.0,
    )
    for kb in range(NKB):
        pt = psX.tile([P, D], DT, tag="x", name="pv")
        nc.tensor.transpose(
            pt[:], vt_sb[:, kb * P : (kb + 1) * P], ident[:D, :D]
        )
        nc.vector.tensor_copy(v_sb[:, kb * (D + 1) : kb * (D + 1) + D], pt[:])

    # --- phase E: attention ---------------------------------------------
    scale = float(D) ** -0.5
    for qp in range(2):
        q0 = qp * 1024
        att = [
            psO.tile([D + 1, 512], DT, name=f"att{qp}_{i}", tag="att")
            for i in range(2)
        ]
        exs = {}
        for kb in range(NKB):
            sc = psX.tile([P, 1024], DT, tag="x", name="sc")
            for i in range(2):
                nc.tensor.matmul(
                    sc[:, i * 512 : (i + 1) * 512],
                    kT[:, kb * P : (kb + 1) * P],
                    qT[:, q0 + i * 512 : q0 + (i + 1) * 512],
                    start=True,
                    stop=True,
                )
            ex = sbE.tile([P, 1024], FDT, name=f"ex{qp}_{kb}", tag="ex")
            nc.scalar.activation(ex[:], sc[:], AF.Exp, scale=scale)
            exs[kb] = ex
            if kb >= 2:
                _att_mms(nc, att, v_sb, exs.pop(kb - 2), kb - 2)
        for kb in (NKB - 2, NKB - 1):
            _att_mms(nc, att, v_sb, exs.pop(kb), kb)

        # --- normalize + batched output for this qb pair -----------------
        ob = sbo.tile([P, 8 * D], DT, tag="ob")
        for i in range(2):
            asb = sbo.tile([D + 1, 512], DT, tag="asb")
            nc.vector.tensor_copy(asb[:], att[i][:])
            for sub in range(4):
                pt = psX.tile([P, D + 1], DT, tag="x", name="pf")
                nc.tensor.transpose(
                    pt[:],
                    asb[:, sub * P : (sub + 1) * P],
                    ident[: D + 1, : D + 1],
                )
                rc = sbo.tile([P, 1], DT, tag="rc")
                nc.vector.reciprocal(rc[:], pt[:, D : D + 1])
                j = i * 4 + sub
                nc.vector.tensor_scalar_mul(
                    ob[:, j * D : (j + 1) * D], pt[:, 0:D], rc[:]
                )
        nc.sync.dma_start(
            out[q0 : q0 + 1024, :].rearrange("(s p) d -> p s d", p=P),
            ob[:].rearrange("p (s d) -> p s d", d=D),
        )


def _att_mms(nc, att, v_sb, ex, kb):
    for i in range(2):
        nc.tensor.matmul(
            att[i][:],
            v_sb[:, kb * (D + 1) : (kb + 1) * (D + 1)],
            ex[:, i * 512 : (i + 1) * 512],
            start=(kb == 0),
            stop=(kb == NKB - 1),
        )


_NC_CACHE = None


def _build():
    global _NC_CACHE
    if _NC_CACHE is not None:
        return _NC_CACHE
    nc = bacc.Bacc(
        "TRN2",
        target_bir_lowering=False,
        debug=False,
        enable_asserts=True,
        num_devices=NCORES,
    )
    x = nc.dram_tensor("x", [NQ, H], DT, kind="ExternalInput").ap()
    Wq = nc.dram_tensor("Wq", [H, D], DT, kind="ExternalInput").ap()
    bq = nc.dram_tensor("bq", [D], DT, kind="ExternalInput").ap()
    Wk = nc.dram_tensor("Wk", [H, D], DT, kind="ExternalInput").ap()
    bk = nc.dram_tensor("bk", [D], DT, kind="ExternalInput").ap()
    Wv = nc.dram_tensor("Wv", [H, D], DT, kind="ExternalInput").ap()
    bv = nc.dram_tensor("bv", [D], DT, kind="ExternalInput").ap()
    out = nc.dram_tensor("out", [NQ, D], DT, kind="ExternalOutput").ap()

    from contextlib import ExitStack

    with tile.TileContext(nc) as tc:
        with ExitStack() as ctx:
            _attention_head(ctx, tc, out, x, (Wq, Wk, Wv), (bq, bk, bv))
    nc.compile()
    _NC_CACHE = nc
    return nc


def _make_in_maps(inputs):
    x = np.ascontiguousarray(np.asarray(inputs["x"], dtype=np.float32))
    small = {
        k: np.ascontiguousarray(np.asarray(inputs[k], dtype=np.float32))
        for k in ("Wq", "bq", "Wk", "bk", "Wv", "bv")
    }
    in_maps = []
    for core in range(NCORES):
        b, h = divmod(core, 2)
        xb = np.ascontiguousarray(x[b, h * NQ : (h + 1) * NQ])
        in_maps.append({"x": xb, **small})
    return in_maps


def _run(inputs, trace=False):
    nc = _build()
    res = run_bass_kernel_spmd(
        nc, _make_in_maps(inputs), core_ids=list(range(NCORES)), trace=trace
    )
    out = np.empty((B, N, D), dtype=np.float32)
    for core in range(NCORES):
        b, h = divmod(core, 2)
        out[b, h * NQ : (h + 1) * NQ] = res.results[core]["out"]
    return out, res


def kernel(**inputs):
    out, _ = _run(inputs, trace=False)
    return out


def _install_ntff_hook():
    """Register the axon NTFF profiling hook that this image's antenv lacks."""
    import types

    try:
        import antenv.axon_hooks  # noqa: F401

        return
    except ImportError:
        pass
    import antenv
    from trn_agent_boot.trn_boot import _ntff_profile_via_ctypes

    import concourse.bass_utils as bu

    mod = types.ModuleType("antenv.axon_hooks")
    _h = [None]
    mod.set_axon_ntff_profile_hook = lambda h: _h.__setitem__(0, h)
    mod.get_axon_ntff_profile_hook = lambda: _h[0]
    sys.modules["antenv.axon_hooks"] = mod
    antenv.axon_hooks = mod
    mod.set_axon_ntff_profile_hook(
        _ntff_profile_via_ctypes("/opt/axon/libaxon_pjrt.so")
    )
    bu.upload_artifacts = lambda tmpdir: tmpdir


def run_traced(inputs):
    _install_ntff_hook()
    out, res = _run(inputs, trace=True)
    return out, res.exec_time_ns


# revision 20
# speedup vs baseline: 1.9309x; 1.0715x over previous
"""Single-head attention (B=4, N=4096, H=768, D=64) on 8 TRN2 NeuronCores.

Sharding: core = (batch b, sequence half h). Each core receives only its
own 2048 rows of x[b], projects Q/K/V for them, then all-gathers the
projected K and V' blocks within the (b,0)/(b,1) pair -- 1 MB of
activations instead of re-loading and re-projecting the partner's 6 MB
of x. Softmax over keys is permutation invariant, so both cores use the
same gathered key order. Output [2048, 64] per core, reassembled host-side.

Matmuls run in float32r (full-rate fp32 streaming, ~1.5e-4 rel err).
The scores contraction (d=64) is padded to K=128 with zero rows: fused
f32r matmuls with K<128 cannot overlap their weight load and run ~1.7x
slower. K and V share one projection matmul (stationary [Wk|Wv], M=128).
Softmax denominators come free from a ones column in the V' blocks.
exp() is batched over two PSUM banks to halve ScalarE per-op overhead.
"""

import sys

sys.path.insert(0, "/opt/trn_rl_repo")

import numpy as np

import concourse.tile as tile
from concourse import bacc, mybir
from concourse.bass_utils import run_bass_kernel_spmd
from concourse.masks import make_identity

B = 4
N = 4096          # keys per batch
NQ = 2048         # queries / own keys per core
H = 768
D = 64
P = 128
HC = H // P       # 6 contraction chunks
NKB = N // P      # 32 key blocks (post-gather)
NOB = NQ // P     # 16 own key blocks
NTB = NQ // 512   # 4 token col-blocks for projections
NCORES = 8

DT = mybir.dt.float32
FDT = mybir.dt.float32r

AF = mybir.ActivationFunctionType


def _attention_head(ctx, tc, out, x, Ws, biases):
    nc = tc.nc
    Wq, Wk, Wv = Ws
    bq, bk, bv = biases

    const = ctx.enter_context(tc.tile_pool(name="const", bufs=1))
    big = ctx.enter_context(tc.tile_pool(name="big", bufs=1))
    xin = ctx.enter_context(tc.tile_pool(name="xin", bufs=3))
    psX = ctx.enter_context(tc.tile_pool(name="psX", bufs=3, space="PSUM"))
    psO = ctx.enter_context(tc.tile_pool(name="psO", bufs=2, space="PSUM"))
    sbE = ctx.enter_context(tc.tile_pool(name="sbE", bufs=4))
    sbo = ctx.enter_context(tc.tile_pool(name="sbo", bufs=4))
    dram = ctx.enter_context(tc.tile_pool(name="dram", bufs=1, space="DRAM"))

    # --- constants (SWDGE queue; keeps the sync queue free for x) ------
    ident = const.tile([P, P], DT)
    make_identity(nc, ident)

    # Tiny warm-up gather: pays the collective launch latency and pair
    # start-skew while the PE is busy with transposes.
    GROUPS = [[2 * i, 2 * i + 1] for i in range(NCORES // 2)]
    warm_in = dram.tile([1, 16], DT)
    warm_out = dram.tile([2, 16], DT)
    nc.gpsimd.dma_start(warm_in[:], ident[0:1, 0:16])
    nc.gpsimd.collective_compute(
        "AllGather",
        mybir.AluOpType.bypass,
        replica_groups=GROUPS,
        ins=[warm_in.opt()],
        outs=[warm_out.opt()],
    )

    w_q = const.tile([P, HC * D], FDT)
    nc.gpsimd.dma_start(
        w_q[:].rearrange("p (c d) -> p c d", d=D),
        Wq.rearrange("(c p) d -> p c d", p=P).bitcast(FDT),
    )
    w_kv = const.tile([P, HC * P], FDT)  # chunk c: [Wk_c | Wv_c]
    for wi, w in enumerate((Wk, Wv)):
        nc.gpsimd.dma_start(
            w_kv[:].rearrange("p (c g) -> p c g", g=P)[:, :, wi * D : (wi + 1) * D],
            w.rearrange("(c p) d -> p c d", p=P).bitcast(FDT),
        )
    bias_sb = const.tile([P, 3], DT)
    for bi, b, r0 in ((0, bq, 0), (1, bk, 0), (2, bv, D)):
        nc.gpsimd.dma_start(
            bias_sb[r0 : r0 + D, bi : bi + 1], b.rearrange("(p f) -> p f", f=1)
        )

    # --- phases A+B interleaved: transpose x, project K/V per 512-token
    # block, and launch a chunked all-gather mid-stream so the collective
    # is fully hidden behind the remaining transposes and Q projection.
    xT = big.tile([P, HC * NQ], FDT)
    kv_sb = big.tile([P, NQ], DT)
    qT = big.tile([P, NQ], FDT)
    nc.gpsimd.memset(qT[D:P, :].bitcast(DT), 0.0)
    kT = big.tile([P, N], FDT)
    nc.gpsimd.memset(kT[D:P, :].bitcast(DT), 0.0)
    vt_sb = big.tile([D, N], DT)
    kv_in = [dram.tile([P, NQ // 2], DT, name=f"kv_in{c}") for c in range(2)]
    kv_out = [
        dram.tile([2 * P, NQ // 2], DT, name=f"kv_out{c}") for c in range(2)
    ]

    def gather_chunk(c):
        nc.scalar.dma_start(
            kv_in[c][:], kv_sb[:, c * 1024 : (c + 1) * 1024]
        )
        nc.gpsimd.collective_compute(
            "AllGather",
            mybir.AluOpType.bypass,
            replica_groups=GROUPS,
            ins=[kv_in[c].opt()],
            outs=[kv_out[c].opt()],
        )

    for tb in range(NTB):
        for nb in range(4 * tb, 4 * tb + 4):
            xt = xin.tile([P, H], DT)
            nc.sync.dma_start(xt[:], x[nb * P : (nb + 1) * P, :])
            for c in range(HC):
                pt = psX.tile([P, P], DT, tag="x")
                nc.tensor.transpose(pt[:], xt[:, c * P : (c + 1) * P], ident[:])
                nc.vector.tensor_copy(
                    xT[:, c * NQ + nb * P : c * NQ + (nb + 1) * P], pt[:]
                )
        s = slice(tb * 512, (tb + 1) * 512)
        ps = psX.tile([P, 512], DT, tag="x", name="pkv")
        for c in range(HC):
            nc.tensor.matmul(
                ps[:],
                w_kv[:, c * P : (c + 1) * P],
                xT[:, c * NQ + tb * 512 : c * NQ + tb * 512 + 512],
                start=(c == 0),
                stop=(c == HC - 1),
            )
        nc.scalar.activation(
            kv_sb[0:D, s], ps[0:D, :], AF.Identity, bias=bias_sb[0:D, 1:2]
        )
        nc.scalar.activation(
            kv_sb[D:P, s], ps[D:P, :], AF.Identity, bias=bias_sb[D:P, 2:3]
        )
        if tb == 1:
            gather_chunk(0)
    gather_chunk(1)

    # --- phase B2: Q projection (overlaps the collectives) --------------
    for tb in range(NTB):
        ps = psX.tile([D, 512], DT, tag="x", name="pq")
        for c in range(HC):
            nc.tensor.matmul(
                ps[:],
                w_q[:, c * D : (c + 1) * D],
                xT[:, c * NQ + tb * 512 : c * NQ + tb * 512 + 512],
                start=(c == 0),
                stop=(c == HC - 1),
            )
        nc.scalar.activation(
            qT[0:D, tb * 512 : (tb + 1) * 512],
            ps[:],
            AF.Identity,
            bias=bias_sb[0:D, 0:1],
        )

    # --- unpack gathered K^T/V^T as each chunk lands --------------------
    # chunk c holds token cols [c*1024, (c+1)*1024) of both pair ranks;
    # gathered key order: [rank0 t0:2048 | rank1 t0:2048].
    for c in range(2):
        for h in range(2):
            col = h * NQ + c * 1024
            nc.scalar.dma_start(
                kT[0:D, col : col + 1024],
                kv_out[c][h * P : h * P + D, :].bitcast(FDT),
            )
            nc.scalar.dma_start(
                vt_sb[:, col : col + 1024],
                kv_out[c][h * P + D : (h + 1) * P, :],
            )

    # --- phase D: V' blocks [128, 65] (col 64 = ones) -------------------
    # key-block order follows chunk arrival: chunk0 covers kb 0-7/16-23.
    KBSEQ = (
        list(range(0, 8))
        + list(range(16, 24))
        + list(range(8, 16))
        + list(range(24, 32))
    )
    v_sb = big.tile([P, NKB * (D + 1)], FDT)
    nc.gpsimd.memset(
        v_sb[:].bitcast(DT).rearrange("p (k c) -> p k c", c=D + 1)[
            :, :, D : D + 1
        ],
        1.0,
    )
    for kb in KBSEQ:
        pt = psX.tile([P, D], DT, tag="x", name="pv")
        nc.tensor.transpose(
            pt[:], vt_sb[:, kb * P : (kb + 1) * P], ident[:D, :D]
        )
        nc.vector.tensor_copy(v_sb[:, kb * (D + 1) : kb * (D + 1) + D], pt[:])

    # --- phase E: attention ---------------------------------------------
    scale = float(D) ** -0.5
    for qp in range(2):
        q0 = qp * 1024
        att = [
            psO.tile([D + 1, 512], DT, name=f"att{qp}_{i}", tag="att")
            for i in range(2)
        ]
        exs = {}
        for pos, kb in enumerate(KBSEQ):
            sc = psX.tile([P, 1024], DT, tag="x", name="sc")
            for i in range(2):
                nc.tensor.matmul(
                    sc[:, i * 512 : (i + 1) * 512],
                    kT[:, kb * P : (kb + 1) * P],
                    qT[:, q0 + i * 512 : q0 + (i + 1) * 512],
                    start=True,
                    stop=True,
                )
            ex = sbE.tile([P, 1024], FDT, name=f"ex{qp}_{kb}", tag="ex")
            nc.scalar.activation(ex[:], sc[:], AF.Exp, scale=scale)
            exs[pos] = ex
            if pos >= 2:
                _att_mms(nc, att, v_sb, exs.pop(pos - 2), KBSEQ[pos - 2], pos - 2)
        for pos in (NKB - 2, NKB - 1):
            _att_mms(nc, att, v_sb, exs.pop(pos), KBSEQ[pos], pos)

        # --- normalize + batched output for this qb pair -----------------
        ob = sbo.tile([P, 8 * D], DT, tag="ob")
        for i in range(2):
            asb = sbo.tile([D + 1, 512], DT, tag="asb")
            nc.vector.tensor_copy(asb[:], att[i][:])
            for sub in range(4):
                pt = psX.tile([P, D + 1], DT, tag="x", name="pf")
                nc.tensor.transpose(
                    pt[:],
                    asb[:, sub * P : (sub + 1) * P],
                    ident[: D + 1, : D + 1],
                )
                rc = sbo.tile([P, 1], DT, tag="rc")
                nc.vector.reciprocal(rc[:], pt[:, D : D + 1])
                j = i * 4 + sub
                nc.vector.tensor_scalar_mul(
                    ob[:, j * D : (j + 1) * D], pt[:, 0:D], rc[:]
                )
        nc.sync.dma_start(
            out[q0 : q0 + 1024, :].rearrange("(s p) d -> p s d", p=P),
            ob[:].rearrange("p (s d) -> p s d", d=D),
        )


def _att_mms(nc, att, v_sb, ex, kb, pos):
    for i in range(2):
        nc.tensor.matmul(
            att[i][:],
            v_sb[:, kb * (D + 1) : (kb + 1) * (D + 1)],
            ex[:, i * 512 : (i + 1) * 512],
            start=(pos == 0),
            stop=(pos == NKB - 1),
        )


_NC_CACHE = None


def _build():
    global _NC_CACHE
    if _NC_CACHE is not None:
        return _NC_CACHE
    nc = bacc.Bacc(
        "TRN2",
        target_bir_lowering=False,
        debug=False,
        enable_asserts=True,
        num_devices=NCORES,
    )
    x = nc.dram_tensor("x", [NQ, H], DT, kind="ExternalInput").ap()
    Wq = nc.dram_tensor("Wq", [H, D], DT, kind="ExternalInput").ap()
    bq = nc.dram_tensor("bq", [D], DT, kind="ExternalInput").ap()
    Wk = nc.dram_tensor("Wk", [H, D], DT, kind="ExternalInput").ap()
    bk = nc.dram_tensor("bk", [D], DT, kind="ExternalInput").ap()
    Wv = nc.dram_tensor("Wv", [H, D], DT, kind="ExternalInput").ap()
    bv = nc.dram_tensor("bv", [D], DT, kind="ExternalInput").ap()
    out = nc.dram_tensor("out", [NQ, D], DT, kind="ExternalOutput").ap()

    from contextlib import ExitStack

    with tile.TileContext(nc) as tc:
        with ExitStack() as ctx:
            _attention_head(ctx, tc, out, x, (Wq, Wk, Wv), (bq, bk, bv))
    nc.compile()
    _NC_CACHE = nc
    return nc


def _make_in_maps(inputs):
    x = np.ascontiguousarray(np.asarray(inputs["x"], dtype=np.float32))
    small = {
        k: np.ascontiguousarray(np.asarray(inputs[k], dtype=np.float32))
        for k in ("Wq", "bq", "Wk", "bk", "Wv", "bv")
    }
    in_maps = []
    for core in range(NCORES):
        b, h = divmod(core, 2)
        xb = np.ascontiguousarray(x[b, h * NQ : (h + 1) * NQ])
        in_maps.append({"x": xb, **small})
    return in_maps


def _run(inputs, trace=False):
    nc = _build()
    res = run_bass_kernel_spmd(
        nc, _make_in_maps(inputs), core_ids=list(range(NCORES)), trace=trace
    )
    out = np.empty((B, N, D), dtype=np.float32)
    for core in range(NCORES):
        b, h = divmod(core, 2)
        out[b, h * NQ : (h + 1) * NQ] = res.results[core]["out"]
    return out, res


def kernel(**inputs):
    out, _ = _run(inputs, trace=False)
    return out


def _install_ntff_hook():
    """Register the axon NTFF profiling hook that this image's antenv lacks."""
    import types

    try:
        import antenv.axon_hooks  # noqa: F401

        return
    except ImportError:
        pass
    import antenv
    from trn_agent_boot.trn_boot import _ntff_profile_via_ctypes

    import concourse.bass_utils as bu

    mod = types.ModuleType("antenv.axon_hooks")
    _h = [None]
    mod.set_axon_ntff_profile_hook = lambda h: _h.__setitem__(0, h)
    mod.get_axon_ntff_profile_hook = lambda: _h[0]
    sys.modules["antenv.axon_hooks"] = mod
    antenv.axon_hooks = mod
    mod.set_axon_ntff_profile_hook(
        _ntff_profile_via_ctypes("/opt/axon/libaxon_pjrt.so")
    )
    bu.upload_artifacts = lambda tmpdir: tmpdir


def run_traced(inputs):
    _install_ntff_hook()
    out, res = _run(inputs, trace=True)
    return out, res.exec_time_ns
